# revision 11
# baseline (speedup 1.0000x reference)
"""2-layer GCN (PyG GCNConv semantics) on 8 Trainium2 NeuronCores.

Distribution: destination-node sharding (12500 nodes/core), edges
partitioned by dst; params replicated; layer-2 input exchanged via a
bf16 AllGather of per-core shards.

Key structure (per core, all matmuls bf16 into fp32 PSUM):
  - Symmetric normalization is factorized: the gather table holds
    dinv[src]-prescaled rows, scatter matrices S are pure 0/1 one-hots,
    and dinv[dst] factors are applied per dst-block in the epilogues
    (relu commutes with the positive diagonal scale; biases enter via
    rank-1 PE matmuls scaled by 1/dinv so results are exact).
  - Edges are grouped per (dst-block, 32768-row table chunk[, parity])
    and packed into 128-slot bins.  Per chunk, one idx stream covers
    all blocks, chopped into 1024-index dma_gather calls (bf16 rows,
    256B each); call tails use negative (skipped) indices.
  - S for the 8 bins of a call is built with ONE DVE tensor_tensor:
    iota pattern vs the per-slot dst_local column broadcast along free.
  - Layer 1: agg[in,dst] += msgsT @ S per bin; z[hid,dst] = W1^T agg
    (+ b1x(1/dinv) rank-1); h = relu (ScalarE, bf16); p[dst,out] =
    h^T W2; rows written x dinv^2 -> bf16 p_shard; AllGather.
  - Layer-2 table is the bf16 p matrix viewed as pair-packed rows
    [GT/2, 128] so gathers stay at the 256B descriptor minimum; bins
    are split by src parity and use the matching 64-wide half of msgs.
  - Layer 2: agg2[dst,out] += S^T @ msgs_half per bin (+ (1/dinv)xb2
    rank-1); written x dinv as fp32 output rows.  No PE transposes.
  - PSUM->SBUF moves ride the idle ScalarE; DVE only builds S and does
    the two per-block scaled writes.

kernel(**inputs) takes FULL inputs, returns the FULL [N, 64] f32 output.
Set GCN_TRACE=1 to capture an NTFF profile (exec time in LAST_EXEC_NS).
"""

import math
import os
import sys
import types

import numpy as np
import ml_dtypes

P = 128
NCORES = 8
CHUNK = 32768          # int16 index range per gather table chunk
CALL = 1024            # idxs per dma_gather call (8 bins)
BINS_PER_CALL = CALL // P


# --------------------------------------------------------------------------
# host-side preprocessing
# --------------------------------------------------------------------------

def _round128(v):
    return ((v + P - 1) // P) * P


def _plan_layer(trows, pars, dst_loc, blks, B, n_tab_rows, npar):
    """Build the uniform call/bin plan plus per-core idx/met arrays.

    trows[c]   : table row index per edge (int64)
    pars[c]    : parity (0..npar-1) per edge, selects the 64-wide half
    dst_loc[c] : dst % 128 per edge
    blks[c]    : dst block per edge
    Returns dict with idx16 [NCORES,128,icols], met [NCORES,128,nbins]
    (bf16), call list and per-block bin lists.
    """
    nchunks = (n_tab_rows + CHUNK - 1) // CHUNK
    G = nchunks * npar
    sizes = np.zeros((NCORES, B, G), np.int64)
    order_by_core = []
    bounds_by_core = []
    for c in range(NCORES):
        ch = trows[c] // CHUNK
        key = blks[c] * G + ch * npar + pars[c]
        order = np.argsort(key, kind="stable")
        key_s = key[order]
        bounds = np.searchsorted(key_s, np.arange(B * G + 1))
        cnt = bounds[1:] - bounds[:-1]
        sizes[c] = cnt.reshape(B, G)
        order_by_core.append(order)
        bounds_by_core.append(bounds)

    seg = np.zeros((B, G), np.int64)
    cap = sizes.max(axis=0)
    for b in range(B):
        for g in range(G):
            seg[b, g] = _round128(int(cap[b, g]))

    # chunk streams: chunk k's stream = concat over b (then parity) of segs
    S_k = [int(seg[:, k * npar:(k + 1) * npar].sum()) for k in range(nchunks)]
    ncalls_k = [(s + CALL - 1) // CALL for s in S_k]
    callbase = np.concatenate([[0], np.cumsum(ncalls_k)]).astype(np.int64)
    # met columns are call-aligned: call J covers bins [8J, 8J+8)
    binbase = callbase * BINS_PER_CALL
    ncalls = int(callbase[-1])
    nbins = int(binbase[-1])
    icols = ncalls * (CALL // 16)

    # stream offset of (b, g) within its chunk
    off = np.zeros((B, G), np.int64)
    run = [0] * nchunks
    for b in range(B):
        for k in range(nchunks):
            for p_ in range(npar):
                g = k * npar + p_
                off[b, g] = run[k]
                run[k] += int(seg[b, g])

    # calls: (chunk, lo, global_call_idx, col0, valid)
    calls = []
    for k in range(nchunks):
        for j in range(ncalls_k[k]):
            J = int(callbase[k]) + j
            v = min(CALL, S_k[k] - j * CALL)
            calls.append(dict(k=k, J=J, col0=J * (CALL // 16),
                              bin0=int(binbase[k]) + j * BINS_PER_CALL,
                              v=int(v)))

    # per-block bin list: (global_bin, chunk, Jglobal, group, parity)
    blocks = []
    for b in range(B):
        bl = []
        for k in range(nchunks):
            for p_ in range(npar):
                g = k * npar + p_
                o = int(off[b, g])
                for s in range(int(seg[b, g]) // P):
                    gk = o // P + s            # bin idx within chunk stream
                    gb = int(binbase[k]) + gk  # global bin
                    J = int(callbase[k]) + gk // BINS_PER_CALL
                    grp = gk % BINS_PER_CALL
                    bl.append((gb, J, grp, p_))
        blocks.append(bl)

    # fill idx16 / met
    idx16 = np.full((NCORES, 16, max(icols, 1)), -1, np.int16)
    met = np.full((NCORES, P, max(nbins, 1)), -1.0, ml_dtypes.bfloat16)
    for c in range(NCORES):
        order = order_by_core[c]
        bounds = bounds_by_core[c]
        tr = trows[c]
        dl = dst_loc[c]
        for b in range(B):
            for k in range(nchunks):
                for p_ in range(npar):
                    g = k * npar + p_
                    s0, s1 = bounds[b * G + g], bounds[b * G + g + 1]
                    n = s1 - s0
                    sg = int(seg[b, g])
                    if sg == 0:
                        continue
                    e = order[s0:s1]
                    if n > 1:
                        e = e[np.argsort(tr[e], kind="stable")]
                    o = int(off[b, g])
                    gb0 = int(binbase[k]) + o // P
                    # slot positions o..o+sg-1 in chunk k's stream
                    iv = np.zeros(sg, np.int64)  # idx values (pad -> 0)
                    if n:
                        iv[:n] = tr[e] - k * CHUNK
                    pos = o + np.arange(sg)
                    idx16[c, pos % 16, int(callbase[k]) * (CALL // 16)
                          + pos // 16] = iv.astype(np.int16)
                    mv = np.full(sg, -1.0, np.float32)
                    if n:
                        mv[:n] = dl[e]
                    met[c, pos % P, gb0 + (np.arange(sg) // P)] = \
                        mv.astype(ml_dtypes.bfloat16)
    # call tails beyond valid stay -1 (skipped by HW); mid-stream pads are 0
    idx_full = np.empty((NCORES, P, max(icols, 1)), np.int16)
    for gsh in range(8):
        idx_full[:, gsh * 16:(gsh + 1) * 16, :] = idx16
    return dict(calls=calls, blocks=blocks, icols=max(icols, 1),
                nbins=max(nbins, 1), idx16=idx_full, met=met,
                nchunks=nchunks, npar=npar)


def _preprocess(x, edge_index):
    N = x.shape[0]
    src = np.concatenate([np.asarray(edge_index[0]), np.arange(N)]).astype(np.int64)
    dst = np.concatenate([np.asarray(edge_index[1]), np.arange(N)]).astype(np.int64)
    deg = np.bincount(dst, minlength=N).astype(np.float64)
    dinv = np.where(deg > 0, 1.0 / np.sqrt(deg), 0.0)

    assert N % NCORES == 0
    NPC = N // NCORES
    B = (NPC + P - 1) // P
    PADN = B * P
    GT = NCORES * PADN

    core_of = dst // NPC
    trows1, trows2, pars2, dstls, blks = [], [], [], [], []
    z = [None] * NCORES
    for c in range(NCORES):
        m = core_of == c
        se = src[m]
        dl = dst[m] - c * NPC
        blks.append(dl // P)
        dstls.append((dl % P).astype(np.float32))
        trows1.append(se)
        spad = (se // NPC) * PADN + (se % NPC)
        trows2.append(spad >> 1)
        pars2.append((spad & 1).astype(np.int64))

    zeros = [np.zeros_like(t) for t in trows1]
    l1 = _plan_layer(trows1, zeros, dstls, blks, B, N, 1)
    l2 = _plan_layer(trows2, pars2, dstls, blks, B, GT // 2, 2)

    # per-core dinv vectors (padded shard layout [128, B])
    dinv_blk = np.zeros((NCORES, P, B), np.float32)
    dinv2_blk = np.zeros((NCORES, P, B), np.float32)
    invd_row = np.zeros((NCORES, 1, PADN), ml_dtypes.bfloat16)
    for c in range(NCORES):
        dv = dinv[c * NPC:(c + 1) * NPC]
        pad = np.zeros(PADN)
        pad[:NPC] = dv
        dinv_blk[c] = pad.reshape(B, P).T.astype(np.float32)
        dinv2_blk[c] = (pad ** 2).reshape(B, P).T.astype(np.float32)
        iv = np.where(pad > 0, 1.0 / np.maximum(pad, 1e-30), 0.0)
        invd_row[c, 0] = iv.astype(ml_dtypes.bfloat16)

    return dict(NPC=NPC, B=B, PADN=PADN, l1=l1, l2=l2, dinv=dinv,
                dinv_blk=dinv_blk, dinv2_blk=dinv2_blk, invd_row=invd_row)


# --------------------------------------------------------------------------
# bass program
# --------------------------------------------------------------------------

def _build(N, IN, HID, OUT, B, PADN, l1, l2, use_b1, use_b2,
           use_collective=True):
    import concourse.bass as bass
    import concourse.bacc as bacc
    import concourse.mybir as mybir
    import concourse.tile as tile

    f32 = mybir.dt.float32
    bf16 = mybir.dt.bfloat16
    i16 = mybir.dt.int16
    i32 = mybir.dt.int32
    eq = mybir.AluOpType.is_equal
    mul = mybir.AluOpType.mult
    Copy = mybir.ActivationFunctionType.Copy
    Relu = mybir.ActivationFunctionType.Relu
    GT = NCORES * PADN

    nc = bacc.Bacc("TRN2", num_devices=NCORES)
    xt = nc.dram_tensor("xt", [N, IN], bf16, kind="ExternalInput")
    idx1 = nc.dram_tensor("idx1", [P, l1["icols"]], i16, kind="ExternalInput")
    idx2 = nc.dram_tensor("idx2", [P, l2["icols"]], i16, kind="ExternalInput")
    met1 = nc.dram_tensor("met1", [P, l1["nbins"]], bf16, kind="ExternalInput")
    met2 = nc.dram_tensor("met2", [P, l2["nbins"]], bf16, kind="ExternalInput")
    w1 = nc.dram_tensor("w1", [IN, HID], bf16, kind="ExternalInput")
    w2 = nc.dram_tensor("w2", [HID, OUT], bf16, kind="ExternalInput")
    b1t = nc.dram_tensor("b1t", [1, HID], bf16, kind="ExternalInput")
    b2t = nc.dram_tensor("b2t", [1, OUT], bf16, kind="ExternalInput")
    dv1 = nc.dram_tensor("dv1", [P, B], f32, kind="ExternalInput")
    dv2 = nc.dram_tensor("dv2", [P, B], f32, kind="ExternalInput")
    ivd = nc.dram_tensor("ivd", [1, PADN], bf16, kind="ExternalInput")
    p_shard = nc.dram_tensor("p_shard", [PADN, OUT], bf16, kind="Internal")
    if use_collective:
        p_full = nc.dram_tensor("p_full", [GT, OUT], bf16, kind="Internal",
                                addr_space="Shared")
    else:
        p_full = nc.dram_tensor("p_full", [GT, OUT], bf16, kind="Internal")
    outt = nc.dram_tensor("outt", [PADN, OUT], f32, kind="ExternalOutput")

    with tile.TileContext(nc) as tc:
        with (
            tc.tile_pool(name="const", bufs=1) as cpool,
            tc.tile_pool(name="meta", bufs=1) as mpool,
            tc.tile_pool(name="gath", bufs=8) as gpool,
            tc.tile_pool(name="smat", bufs=8) as spool,
            tc.tile_pool(name="work", bufs=4) as wpool,
            tc.tile_pool(name="psA", bufs=2, space="PSUM") as psA,
            tc.tile_pool(name="psB", bufs=2, space="PSUM") as psB,
            tc.tile_pool(name="psC", bufs=2, space="PSUM") as psC,
        ):
            w1_sb = cpool.tile([IN, HID], bf16)
            nc.sync.dma_start(w1_sb[:], w1[:])
            w2_sb = cpool.tile([HID, OUT], bf16)
            nc.sync.dma_start(w2_sb[:], w2[:])
            b1_sb = cpool.tile([1, HID], bf16)
            nc.sync.dma_start(b1_sb[:], b1t[:])
            b2_sb = cpool.tile([1, OUT], bf16)
            nc.sync.dma_start(b2_sb[:], b2t[:])
            dv1_sb = cpool.tile([P, B], f32)
            nc.sync.dma_start(dv1_sb[:], dv1[:])
            dv2_sb = cpool.tile([P, B], f32)
            nc.sync.dma_start(dv2_sb[:], dv2[:])
            ivd_sb = cpool.tile([1, PADN], bf16)
            nc.sync.dma_start(ivd_sb[:], ivd[:])

            iota_i = cpool.tile([P, CALL], i32)
            nc.gpsimd.iota(iota_i[:], pattern=[[0, BINS_PER_CALL], [1, P]],
                           base=0, channel_multiplier=0)
            iota8 = cpool.tile([P, CALL], bf16)
            nc.vector.tensor_copy(iota8[:], iota_i[:])

            idx1_sb = mpool.tile([P, l1["icols"]], i16)
            nc.sync.dma_start(idx1_sb[:], idx1[:])
            met1_sb = mpool.tile([P, l1["nbins"]], bf16)
            nc.sync.dma_start(met1_sb[:], met1[:])
            idx2_sb = mpool.tile([P, l2["icols"]], i16)
            nc.sync.dma_start(idx2_sb[:], idx2[:])
            met2_sb = mpool.tile([P, l2["nbins"]], bf16)
            nc.sync.dma_start(met2_sb[:], met2[:])

            rows_p = mpool.tile([P, B * OUT], bf16)
            rows_o = mpool.tile([P, B * OUT], f32)

            def run_layer(lp, table_ap, tab_rows, idx_sb, met_sb, elem,
                          epilogue):
                """Emit gathers/S-builds on demand and per-block matmul
                accumulation; epilogue(b, agg_ps) per block."""
                tiles = {}     # J -> (msgs_tile, S_tile)
                emitted = [0] * lp["nchunks"]
                callbase = {}
                chunk_of = {}
                for cinfo in lp["calls"]:
                    callbase.setdefault(cinfo["k"], []).append(cinfo)
                    chunk_of[cinfo["J"]] = cinfo["k"]

                def emit_call(k, jloc):
                    cinfo = callbase[k][jloc]
                    J = cinfo["J"]
                    lo = k * CHUNK
                    hi = min(lo + CHUNK, tab_rows)
                    msgs = gpool.tile([P, BINS_PER_CALL * elem], bf16,
                                      tag="msgs")
                    nc.gpsimd.dma_gather(
                        out_ap=msgs[:].rearrange("p (s e) -> p s e", e=elem),
                        in_ap=table_ap[lo:hi],
                        idxs_ap=idx_sb[:, cinfo["col0"]:cinfo["col0"] + CALL // 16],
                        num_idxs=CALL,
                        num_idxs_reg=cinfo["v"],
                        elem_size=elem,
                        single_packet=False,
                    )
                    S = spool.tile([P, CALL], bf16, tag="S")
                    met_b = met_sb[:, cinfo["bin0"]:cinfo["bin0"] + BINS_PER_CALL]
                    nc.vector.tensor_tensor(
                        S[:].rearrange("p (k f) -> p k f", f=P),
                        iota8[:].rearrange("p (k f) -> p k f", f=P),
                        met_b.unsqueeze(2).broadcast_to([P, BINS_PER_CALL, P]),
                        eq,
                    )
                    tiles[J] = (msgs, S)

                for b in range(B):
                    bl = lp["blocks"][b]
                    agg_ps = None
                    nb = len(bl)
                    for i, (gb, J, grp, par) in enumerate(bl):
                        k = chunk_of[J]
                        jloc = J - callbase[k][0]["J"]
                        while emitted[k] <= jloc:
                            emit_call(k, emitted[k])
                            emitted[k] += 1
                        msgs, S = tiles[J]
                        if agg_ps is None:
                            agg_ps = epilogue.new_psum()
                        epilogue.matmul(agg_ps, msgs, S, grp, par,
                                        start=(i == 0), stop=(i == nb - 1))
                    epilogue.finish(b, agg_ps)

            # ---------------- layer 1 ----------------
            class Epi1:
                def new_psum(self):
                    return psA.tile([IN, P], f32, tag="agg", name="agg_ps")

                def matmul(self, agg_ps, msgs, S, grp, par, start, stop):
                    nc.tensor.matmul(
                        agg_ps[:],
                        lhsT=msgs[:, grp * IN:(grp + 1) * IN],
                        rhs=S[:, grp * P:(grp + 1) * P],
                        start=start, stop=stop,
                    )

                def finish(self, b, agg_ps):
                    agg_sb = wpool.tile([IN, P], bf16, tag="aggsb")
                    if agg_ps is None:
                        nc.vector.memset(agg_sb[:], 0.0)
                    else:
                        nc.scalar.activation(agg_sb[:], agg_ps[:], Copy)
                    z_ps = psB.tile([HID, P], f32, tag="z")
                    nc.tensor.matmul(z_ps[:], lhsT=w1_sb[:], rhs=agg_sb[:],
                                     start=True, stop=not use_b1)
                    if use_b1:
                        nc.tensor.matmul(
                            z_ps[:], lhsT=b1_sb[:],
                            rhs=ivd_sb[:, b * P:(b + 1) * P],
                            start=False, stop=True,
                        )
                    h_sb = wpool.tile([HID, P], bf16, tag="h")
                    nc.scalar.activation(h_sb[:], z_ps[:], Relu)
                    p_ps = psC.tile([P, OUT], f32, tag="p")
                    nc.tensor.matmul(p_ps[:], lhsT=h_sb[:], rhs=w2_sb[:],
                                     start=True, stop=True)
                    nc.vector.tensor_scalar(
                        rows_p[:, b * OUT:(b + 1) * OUT], p_ps[:],
                        dv2_sb[:, b:b + 1], None, mul,
                    )

            run_layer(l1, xt[:], N, idx1_sb, met1_sb, IN, Epi1())

            nc.sync.dma_start(p_shard[:].rearrange("(b p) f -> p b f", p=P),
                              rows_p[:])
            if use_collective:
                nc.gpsimd.collective_compute(
                    "AllGather",
                    mybir.AluOpType.bypass,
                    replica_groups=[list(range(NCORES))],
                    ins=[p_shard[:]],
                    outs=[p_full[:]],
                )
            else:
                nc.sync.dma_start(p_full[0:PADN, :], p_shard[:])

            # ---------------- layer 2 ----------------
            tab2 = p_full[:].rearrange("(r two) f -> r (two f)", two=2)

            class Epi2Driver:
                def __init__(self):
                    self.b = 0

                def new_psum(self):
                    return psC.tile([P, OUT], f32, tag="agg2", name="agg2_ps")

                def matmul(self, agg_ps, msgs, S, grp, par, start, stop):
                    nc.tensor.matmul(
                        agg_ps[:],
                        lhsT=S[:, grp * P:(grp + 1) * P],
                        rhs=msgs[:, grp * 2 * OUT + par * OUT:
                                 grp * 2 * OUT + (par + 1) * OUT],
                        start=start, stop=stop and not use_b2,
                    )

                def finish(self, b, agg_ps):
                    if agg_ps is None:
                        nc.vector.memset(rows_o[:, b * OUT:(b + 1) * OUT], 0.0)
                        return
                    if use_b2:
                        nc.tensor.matmul(
                            agg_ps[:],
                            lhsT=ivd_sb[:, b * P:(b + 1) * P],
                            rhs=b2_sb[:],
                            start=False, stop=True,
                        )
                    nc.vector.tensor_scalar(
                        rows_o[:, b * OUT:(b + 1) * OUT], agg_ps[:],
                        dv1_sb[:, b:b + 1], None, mul,
                    )

            run_layer(l2, tab2, GT // 2, idx2_sb, met2_sb, 2 * OUT,
                      Epi2Driver())

            nc.sync.dma_start(outt[:].rearrange("(b p) f -> p b f", p=P),
                              rows_o[:])

    nc.compile()
    return nc


# --------------------------------------------------------------------------
# optional NTFF tracing (dev only; registers the axon profile hook)
# --------------------------------------------------------------------------

def _install_trace_shim():
    try:
        if "antenv.axon_hooks" in sys.modules:
            return True
        import antenv

        mod = types.ModuleType("antenv.axon_hooks")
        mod._hook = None
        mod.set_axon_ntff_profile_hook = lambda h: setattr(mod, "_hook", h)
        mod.get_axon_ntff_profile_hook = lambda: mod._hook
        sys.modules["antenv.axon_hooks"] = mod
        antenv.axon_hooks = mod
        from trn_agent_boot.trn_boot import _ntff_profile_via_ctypes

        mod.set_axon_ntff_profile_hook(
            _ntff_profile_via_ctypes("/opt/axon/libaxon_pjrt.so")
        )
        import concourse.bass_utils as bu

        bu.upload_artifacts = lambda tmpdir: ""
        return True
    except Exception:
        return False


LAST_EXEC_NS = None
LAST_RESULTS = None


def kernel(x, edge_index, W1, b1, W2, b2):
    global LAST_EXEC_NS, LAST_RESULTS
    from concourse.bass_utils import run_bass_kernel_spmd

    x = np.asarray(x, dtype=np.float32)
    W1 = np.ascontiguousarray(np.asarray(W1, np.float32))
    b1 = np.asarray(b1, np.float32)
    W2 = np.ascontiguousarray(np.asarray(W2, np.float32))
    b2 = np.asarray(b2, np.float32)
    N, IN = x.shape
    HID = W1.shape[1]
    OUT = W2.shape[1]

    pp = _preprocess(x, edge_index)
    B, PADN, NPC = pp["B"], pp["PADN"], pp["NPC"]
    use_b1 = bool(np.any(b1))
    use_b2 = bool(np.any(b2))

    nc = _build(N, IN, HID, OUT, B, PADN, pp["l1"], pp["l2"],
                use_b1, use_b2)

    xs = (x * pp["dinv"][:, None].astype(np.float32)).astype(ml_dtypes.bfloat16)
    xs = np.ascontiguousarray(xs)
    w1b = W1.astype(ml_dtypes.bfloat16)
    w2b = W2.astype(ml_dtypes.bfloat16)
    b1b = b1.reshape(1, HID).astype(ml_dtypes.bfloat16)
    b2b = b2.reshape(1, OUT).astype(ml_dtypes.bfloat16)

    in_maps = []
    for c in range(NCORES):
        in_maps.append(
            {
                "xt": xs,
                "idx1": pp["l1"]["idx16"][c],
                "idx2": pp["l2"]["idx16"][c],
                "met1": pp["l1"]["met"][c],
                "met2": pp["l2"]["met"][c],
                "w1": w1b,
                "w2": w2b,
                "b1t": b1b,
                "b2t": b2b,
                "dv1": pp["dinv_blk"][c],
                "dv2": pp["dinv2_blk"][c],
                "ivd": pp["invd_row"][c],
            }
        )

    trace = bool(int(os.environ.get("GCN_TRACE", "0")))
    if trace:
        trace = _install_trace_shim()
    res = run_bass_kernel_spmd(
        nc, in_maps, core_ids=list(range(NCORES)), trace=trace
    )
    LAST_EXEC_NS = res.exec_time_ns
    LAST_RESULTS = res

    out = np.empty((N, OUT), np.float32)
    for c in range(NCORES):
        out[c * NPC:(c + 1) * NPC] = res.results[c]["outt"][:NPC]
    return out


# revision 25
# speedup vs baseline: 1.4282x; 1.4282x over previous
"""2-layer GCN (PyG GCNConv semantics) on 8 Trainium2 NeuronCores.

Distribution: destination-node sharding (12500 nodes/core), edges
partitioned by dst; params replicated; layer-2 input exchanged via a
bf16 AllGather of per-core shards.

Key structure (per core, all matmuls bf16 into fp32 PSUM):
  - Symmetric normalization is factorized: the gather table holds
    dinv[src]-prescaled rows, scatter matrices S are pure 0/1 one-hots,
    and dinv[dst] factors are applied per dst-block in the epilogues
    (relu commutes with the positive diagonal scale; biases enter via
    rank-1 PE matmuls scaled by 1/dinv so results are exact).
  - Edges are grouped per (dst-block, 32768-row table chunk[, parity])
    and packed into 128-slot bins.  Per chunk, one idx stream covers
    all blocks, chopped into 1024-index dma_gather calls (bf16 rows,
    256B each); call tails use negative (skipped) indices.
  - S for the 8 bins of a call is built with ONE DVE tensor_tensor:
    iota pattern vs the per-slot dst_local column broadcast along free.
  - Layer 1: agg[in,dst] += msgsT @ S per bin; z[hid,dst] = W1^T agg
    (+ b1x(1/dinv) rank-1); h = relu (ScalarE, bf16); p[dst,out] =
    h^T W2; rows written x dinv^2 -> bf16 p_shard; AllGather.
  - Layer-2 table is the bf16 p matrix viewed as pair-packed rows
    [GT/2, 128] so gathers stay at the 256B descriptor minimum; bins
    are split by src parity and use the matching 64-wide half of msgs.
  - Layer 2: agg2[dst,out] += S^T @ msgs_half per bin (+ (1/dinv)xb2
    rank-1); written x dinv as fp32 output rows.  No PE transposes.
  - PSUM->SBUF moves ride the idle ScalarE; DVE only builds S and does
    the two per-block scaled writes.

kernel(**inputs) takes FULL inputs, returns the FULL [N, 64] f32 output.
Set GCN_TRACE=1 to capture an NTFF profile (exec time in LAST_EXEC_NS).
"""

import math
import os
import sys
import types

import numpy as np
import ml_dtypes

P = 128
NCORES = 8
CHUNK = 32768          # int16 index range limit per gather table chunk
CALL = 2048            # idxs per dma_gather call (16 bins)
BINS_PER_CALL = CALL // P


# --------------------------------------------------------------------------
# host-side preprocessing
# --------------------------------------------------------------------------

def _round128(v):
    return ((v + P - 1) // P) * P


def _plan_layer(trows, pars, dst_loc, blks, B, n_tab_rows, npar):
    """Build the uniform call/bin plan plus per-core idx/met arrays.

    trows[c]   : table row index per edge (int64)
    pars[c]    : parity (0..npar-1) per edge, selects the 64-wide half
    dst_loc[c] : dst % 128 per edge
    blks[c]    : dst block per edge
    Returns dict with idx16 [NCORES,128,icols], met [NCORES,128,nbins]
    (bf16), call list and per-block bin lists.
    """
    nchunks = (n_tab_rows + CHUNK - 1) // CHUNK
    cw = (n_tab_rows + nchunks - 1) // nchunks  # equal-width chunks
    G = nchunks * npar
    sizes = np.zeros((NCORES, B, G), np.int64)
    order_by_core = []
    bounds_by_core = []
    for c in range(NCORES):
        ch = trows[c] // cw
        key = blks[c] * G + ch * npar + pars[c]
        order = np.argsort(key, kind="stable")
        key_s = key[order]
        bounds = np.searchsorted(key_s, np.arange(B * G + 1))
        cnt = bounds[1:] - bounds[:-1]
        sizes[c] = cnt.reshape(B, G)
        order_by_core.append(order)
        bounds_by_core.append(bounds)

    # exact segments (no rounding): boundary bins span two segments and
    # get one scatter-matmul instance per segment touching them
    seg = sizes.max(axis=0)  # [B, G] caps

    S_k = [int(seg[:, k * npar:(k + 1) * npar].sum()) for k in range(nchunks)]
    ncalls_k = [(s + CALL - 1) // CALL for s in S_k]
    callbase = np.concatenate([[0], np.cumsum(ncalls_k)]).astype(np.int64)
    ncalls = int(callbase[-1])
    icols = ncalls * (CALL // 16)

    # stream offset of (b, g) within its chunk
    off = np.zeros((B, G), np.int64)
    run = [0] * nchunks
    for b in range(B):
        for k in range(nchunks):
            for p_ in range(npar):
                g = k * npar + p_
                off[b, g] = run[k]
                run[k] += int(seg[b, g])

    # instances: per (b, g) one per touched 128-slot group, in stream
    # order (== group order per chunk).  Assign met columns sequentially
    # per call.
    inst_of = {}              # (b, g) -> list of (J, grp_in_call, met_col)
    n_inst_call = [0] * ncalls
    inst0_call = [0] * ncalls
    per_chunk_insts = [[] for _ in range(nchunks)]
    for k in range(nchunks):
        items = []
        for b in range(B):
            for p_ in range(npar):
                g = k * npar + p_
                cap = int(seg[b, g])
                if cap == 0:
                    inst_of[(b, g)] = []
                    continue
                o = int(off[b, g])
                g0, g1 = o // P, (o + cap - 1) // P
                items.append((o, b, g, g0, g1))
        items.sort()
        for (o, b, g, g0, g1) in items:
            lst = []
            for grp in range(g0, g1 + 1):
                J = int(callbase[k]) + grp // BINS_PER_CALL
                lst.append([J, grp % BINS_PER_CALL, None])
                per_chunk_insts[k].append((grp, o, b, g, lst[-1]))
            inst_of[(b, g)] = lst
    # met columns: order instances by (call, then stream order)
    nmet = 0
    for k in range(nchunks):
        per_chunk_insts[k].sort(key=lambda t: (t[0], t[1]))
        for (grp, o, b, g, ref) in per_chunk_insts[k]:
            J = ref[0]
            n_inst_call[J] += 1
    for J in range(1, ncalls):
        inst0_call[J] = inst0_call[J - 1] + n_inst_call[J - 1]
    nmet = inst0_call[-1] + n_inst_call[-1] if ncalls else 0
    fill = [0] * ncalls
    for k in range(nchunks):
        for (grp, o, b, g, ref) in per_chunk_insts[k]:
            J = ref[0]
            ref[2] = inst0_call[J] + fill[J]
            fill[J] += 1

    calls = []
    for k in range(nchunks):
        for j in range(ncalls_k[k]):
            J = int(callbase[k]) + j
            v = min(CALL, S_k[k] - j * CALL)
            calls.append(dict(k=k, J=J, col0=J * (CALL // 16), v=int(v),
                              inst0=inst0_call[J], n_inst=n_inst_call[J]))

    # per-block instance list in stream order
    blocks = []
    for b in range(B):
        bl = []
        for k in range(nchunks):
            for p_ in range(npar):
                g = k * npar + p_
                for (J, grp, col) in inst_of[(b, g)]:
                    bl.append((J, grp, col, p_))
        blocks.append(bl)

    # fill idx16 / met
    idx16 = np.full((NCORES, 16, max(icols, 1)), -1, np.int16)
    met = np.full((NCORES, P, max(nmet, 1)), -1.0, ml_dtypes.bfloat16)
    for c in range(NCORES):
        order = order_by_core[c]
        bounds = bounds_by_core[c]
        tr = trows[c]
        dl = dst_loc[c]
        for b in range(B):
            for k in range(nchunks):
                for p_ in range(npar):
                    g = k * npar + p_
                    s0, s1 = bounds[b * G + g], bounds[b * G + g + 1]
                    n = s1 - s0
                    cap = int(seg[b, g])
                    if cap == 0:
                        continue
                    e = order[s0:s1]
                    if n > 1:
                        e = e[np.argsort(tr[e], kind="stable")]
                    o = int(off[b, g])
                    iv = np.zeros(cap, np.int64)  # idx values (pad -> 0)
                    if n:
                        iv[:n] = tr[e] - k * cw
                    pos = o + np.arange(cap)
                    idx16[c, pos % 16, int(callbase[k]) * (CALL // 16)
                          + pos // 16] = iv.astype(np.int16)
                    mv = np.full(cap, -1.0, np.float32)
                    if n:
                        mv[:n] = dl[e]
                    # met column of slot = instance of its group
                    g0 = o // P
                    cols = np.array([ic[2] for ic in inst_of[(b, g)]],
                                    np.int64)
                    met[c, pos % P, cols[pos // P - g0]] = \
                        mv.astype(ml_dtypes.bfloat16)
    # call tails beyond valid stay -1 (skipped by HW); mid-stream pads are 0
    idx_full = np.empty((NCORES, P, max(icols, 1)), np.int16)
    for gsh in range(8):
        idx_full[:, gsh * 16:(gsh + 1) * 16, :] = idx16
    return dict(calls=calls, blocks=blocks, icols=max(icols, 1),
                nbins=max(nmet, 1), idx16=idx_full, met=met,
                nchunks=nchunks, npar=npar, cw=cw)


def _preprocess(x, edge_index):
    N = x.shape[0]
    src = np.concatenate([np.asarray(edge_index[0]), np.arange(N)]).astype(np.int64)
    dst = np.concatenate([np.asarray(edge_index[1]), np.arange(N)]).astype(np.int64)
    deg = np.bincount(dst, minlength=N).astype(np.float64)
    dinv = np.where(deg > 0, 1.0 / np.sqrt(deg), 0.0)

    assert N % NCORES == 0
    NPC = N // NCORES
    B = (NPC + P - 1) // P
    PADN = B * P
    GT = NCORES * PADN
    NB = NCORES * B

    # Degree-balanced dst assignment: deal nodes round-robin (by degree
    # rank) across the NCORES*B (core, block) bins so per-bin edge counts
    # are nearly equal across cores -> minimal cap-max padding.
    order = np.argsort(-deg, kind="stable")
    newpos = np.empty(N, np.int64)
    ranks = np.arange(N)
    newpos[order] = (ranks % NB) * P + ranks // NB
    assert newpos.max() < GT

    dnew = newpos[dst]
    core_of = dnew // PADN
    trows1, trows2, pars2, dstls, blks = [], [], [], [], []
    for c in range(NCORES):
        m = core_of == c
        se = src[m]
        dl = dnew[m] - c * PADN
        blks.append(dl // P)
        dstls.append((dl % P).astype(np.float32))
        trows1.append(se)
        spad = newpos[se]
        trows2.append(spad >> 1)
        pars2.append((spad & 1).astype(np.int64))

    zeros = [np.zeros_like(t) for t in trows1]
    l1 = _plan_layer(trows1, zeros, dstls, blks, B, N, 1)
    l2 = _plan_layer(trows2, pars2, dstls, blks, B, GT // 2, 2)

    # per-core dinv vectors in the permuted padded layout [128, B]
    dinv_pad = np.zeros(GT)
    dinv_pad[newpos] = dinv
    dinv_blk = np.zeros((NCORES, P, B), np.float32)
    dinv2_blk = np.zeros((NCORES, P, B), np.float32)
    invd_row = np.zeros((NCORES, 1, PADN), ml_dtypes.bfloat16)
    for c in range(NCORES):
        pad = dinv_pad[c * PADN:(c + 1) * PADN]
        dinv_blk[c] = pad.reshape(B, P).T.astype(np.float32)
        dinv2_blk[c] = (pad ** 2).reshape(B, P).T.astype(np.float32)
        iv = np.where(pad > 0, 1.0 / np.maximum(pad, 1e-30), 0.0)
        invd_row[c, 0] = iv.astype(ml_dtypes.bfloat16)

    return dict(NPC=NPC, B=B, PADN=PADN, l1=l1, l2=l2, dinv=dinv,
                newpos=newpos, dinv_blk=dinv_blk, dinv2_blk=dinv2_blk,
                invd_row=invd_row)


# --------------------------------------------------------------------------
# bass program
# --------------------------------------------------------------------------

def _build(N, IN, HID, OUT, B, PADN, l1, l2, use_b1, use_b2,
           use_collective=True):
    import concourse.bass as bass
    import concourse.bacc as bacc
    import concourse.mybir as mybir
    import concourse.tile as tile

    f32 = mybir.dt.float32
    bf16 = mybir.dt.bfloat16
    i16 = mybir.dt.int16
    i32 = mybir.dt.int32
    eq = mybir.AluOpType.is_equal
    mul = mybir.AluOpType.mult
    Copy = mybir.ActivationFunctionType.Copy
    Relu = mybir.ActivationFunctionType.Relu
    GT = NCORES * PADN

    nc = bacc.Bacc("TRN2", num_devices=NCORES)
    xt = nc.dram_tensor("xt", [N, IN], bf16, kind="ExternalInput")
    idx1 = nc.dram_tensor("idx1", [P, l1["icols"]], i16, kind="ExternalInput")
    idx2 = nc.dram_tensor("idx2", [P, l2["icols"]], i16, kind="ExternalInput")
    met1 = nc.dram_tensor("met1", [P, l1["nbins"]], bf16, kind="ExternalInput")
    met2 = nc.dram_tensor("met2", [P, l2["nbins"]], bf16, kind="ExternalInput")
    w1 = nc.dram_tensor("w1", [IN, HID], bf16, kind="ExternalInput")
    w2 = nc.dram_tensor("w2", [HID, OUT], bf16, kind="ExternalInput")
    b1t = nc.dram_tensor("b1t", [1, HID], bf16, kind="ExternalInput")
    b2t = nc.dram_tensor("b2t", [1, OUT], bf16, kind="ExternalInput")
    dv1 = nc.dram_tensor("dv1", [P, B], f32, kind="ExternalInput")
    dv2 = nc.dram_tensor("dv2", [P, B], f32, kind="ExternalInput")
    ivd = nc.dram_tensor("ivd", [1, PADN], bf16, kind="ExternalInput")
    p_shard = nc.dram_tensor("p_shard", [PADN, OUT], bf16, kind="Internal")
    if use_collective:
        p_full = nc.dram_tensor("p_full", [GT, OUT], bf16, kind="Internal",
                                addr_space="Shared")
    else:
        p_full = nc.dram_tensor("p_full", [GT, OUT], bf16, kind="Internal")
    outt = nc.dram_tensor("outt", [PADN, OUT], f32, kind="ExternalOutput")

    with tile.TileContext(nc) as tc:
        with (
            tc.tile_pool(name="const", bufs=1) as cpool,
            tc.tile_pool(name="meta", bufs=1) as mpool,
            tc.tile_pool(name="gath", bufs=6) as gpool,
            tc.tile_pool(name="smat", bufs=6) as spool,
            tc.tile_pool(name="work", bufs=4) as wpool,
            tc.tile_pool(name="psA", bufs=2, space="PSUM") as psA,
            tc.tile_pool(name="psB", bufs=2, space="PSUM") as psB,
            tc.tile_pool(name="psC", bufs=2, space="PSUM") as psC,
        ):
            w1_sb = cpool.tile([IN, HID], bf16)
            nc.sync.dma_start(w1_sb[:], w1[:])
            w2_sb = cpool.tile([HID, OUT], bf16)
            nc.sync.dma_start(w2_sb[:], w2[:])
            b1_sb = cpool.tile([1, HID], bf16)
            nc.sync.dma_start(b1_sb[:], b1t[:])
            b2_sb = cpool.tile([1, OUT], bf16)
            nc.sync.dma_start(b2_sb[:], b2t[:])
            dv1_sb = cpool.tile([P, B], f32)
            nc.sync.dma_start(dv1_sb[:], dv1[:])
            dv2_sb = cpool.tile([P, B], f32)
            nc.sync.dma_start(dv2_sb[:], dv2[:])
            ivd_sb = cpool.tile([1, PADN], bf16)
            nc.sync.dma_start(ivd_sb[:], ivd[:])

            maxi = max(max(c["n_inst"] for c in l1["calls"]),
                       max(c["n_inst"] for c in l2["calls"]))
            iota_i = cpool.tile([P, maxi * P], i32)
            nc.gpsimd.iota(iota_i[:], pattern=[[0, maxi], [1, P]],
                           base=0, channel_multiplier=0)
            iota8 = cpool.tile([P, maxi * P], bf16)
            nc.vector.tensor_copy(iota8[:], iota_i[:])

            idx1_sb = mpool.tile([P, l1["icols"]], i16)
            nc.sync.dma_start(idx1_sb[:], idx1[:])
            met1_sb = mpool.tile([P, l1["nbins"]], bf16)
            nc.sync.dma_start(met1_sb[:], met1[:])
            idx2_sb = mpool.tile([P, l2["icols"]], i16)
            nc.sync.dma_start(idx2_sb[:], idx2[:])
            met2_sb = mpool.tile([P, l2["nbins"]], bf16)
            nc.sync.dma_start(met2_sb[:], met2[:])

            rows_p = mpool.tile([P, B * OUT], bf16)
            rows_o = mpool.tile([P, B * OUT], f32)

            def run_layer(lp, table_ap, tab_rows, idx_sb, met_sb, elem,
                          epilogue):
                """Emit gathers/S-builds on demand and per-block matmul
                accumulation; epilogue(b, agg_ps) per block."""
                tiles = {}     # J -> (msgs_tile, S_tile)
                emitted = [0] * lp["nchunks"]
                callbase = {}
                by_J = {}
                for cinfo in lp["calls"]:
                    callbase.setdefault(cinfo["k"], []).append(cinfo)
                    by_J[cinfo["J"]] = cinfo

                def emit_call(k, jloc):
                    cinfo = callbase[k][jloc]
                    J = cinfo["J"]
                    lo = k * lp["cw"]
                    hi = min(lo + lp["cw"], tab_rows)
                    msgs = gpool.tile([P, BINS_PER_CALL * elem], bf16,
                                      tag="msgs")
                    nc.gpsimd.dma_gather(
                        out_ap=msgs[:].rearrange("p (s e) -> p s e", e=elem),
                        in_ap=table_ap[lo:hi],
                        idxs_ap=idx_sb[:, cinfo["col0"]:cinfo["col0"] + CALL // 16],
                        num_idxs=CALL,
                        num_idxs_reg=cinfo["v"],
                        elem_size=elem,
                        single_packet=False,
                    )
                    ni = cinfo["n_inst"]
                    S = spool.tile([P, ni * P], bf16, tag="S")
                    met_b = met_sb[:, cinfo["inst0"]:cinfo["inst0"] + ni]
                    nc.vector.tensor_tensor(
                        S[:].rearrange("p (k f) -> p k f", f=P),
                        iota8[:, :ni * P].rearrange("p (k f) -> p k f", f=P),
                        met_b.unsqueeze(2).broadcast_to([P, ni, P]),
                        eq,
                    )
                    tiles[J] = (msgs, S)

                for b in range(B):
                    bl = lp["blocks"][b]
                    agg_ps = None
                    nb = len(bl)
                    for i, (J, grp, col, par) in enumerate(bl):
                        cinfo = by_J[J]
                        k = cinfo["k"]
                        jloc = J - callbase[k][0]["J"]
                        while emitted[k] <= jloc:
                            emit_call(k, emitted[k])
                            emitted[k] += 1
                        msgs, S = tiles[J]
                        if agg_ps is None:
                            agg_ps = epilogue.new_psum()
                        epilogue.matmul(agg_ps, msgs, S, grp,
                                        col - cinfo["inst0"], par,
                                        start=(i == 0), stop=(i == nb - 1))
                    epilogue.finish(b, agg_ps)

            # ---------------- layer 1 ----------------
            class Epi1:
                def new_psum(self):
                    return psA.tile([IN, P], f32, tag="agg", name="agg_ps")

                def matmul(self, agg_ps, msgs, S, grp, sloc, par, start, stop):
                    nc.tensor.matmul(
                        agg_ps[:],
                        lhsT=msgs[:, grp * IN:(grp + 1) * IN],
                        rhs=S[:, sloc * P:(sloc + 1) * P],
                        start=start, stop=stop,
                    )

                def finish(self, b, agg_ps):
                    agg_sb = wpool.tile([IN, P], bf16, tag="aggsb")
                    if agg_ps is None:
                        nc.vector.memset(agg_sb[:], 0.0)
                    else:
                        nc.scalar.activation(agg_sb[:], agg_ps[:], Copy)
                    z_ps = psB.tile([HID, P], f32, tag="z")
                    nc.tensor.matmul(z_ps[:], lhsT=w1_sb[:], rhs=agg_sb[:],
                                     start=True, stop=not use_b1)
                    if use_b1:
                        nc.tensor.matmul(
                            z_ps[:], lhsT=b1_sb[:],
                            rhs=ivd_sb[:, b * P:(b + 1) * P],
                            start=False, stop=True,
                        )
                    h_sb = wpool.tile([HID, P], bf16, tag="h")
                    nc.scalar.activation(h_sb[:], z_ps[:], Relu)
                    p_ps = psC.tile([P, OUT], f32, tag="p")
                    nc.tensor.matmul(p_ps[:], lhsT=h_sb[:], rhs=w2_sb[:],
                                     start=True, stop=True)
                    nc.vector.tensor_scalar(
                        rows_p[:, b * OUT:(b + 1) * OUT], p_ps[:],
                        dv2_sb[:, b:b + 1], None, mul,
                    )

            run_layer(l1, xt[:], N, idx1_sb, met1_sb, IN, Epi1())

            nc.sync.dma_start(p_shard[:].rearrange("(b p) f -> p b f", p=P),
                              rows_p[:])
            if use_collective:
                nc.gpsimd.collective_compute(
                    "AllGather",
                    mybir.AluOpType.bypass,
                    replica_groups=[list(range(NCORES))],
                    ins=[p_shard[:]],
                    outs=[p_full[:]],
                )
            else:
                nc.sync.dma_start(p_full[0:PADN, :], p_shard[:])

            # ---------------- layer 2 ----------------
            tab2 = p_full[:].rearrange("(r two) f -> r (two f)", two=2)

            class Epi2Driver:
                def __init__(self):
                    self.b = 0

                def new_psum(self):
                    return psC.tile([P, OUT], f32, tag="agg2", name="agg2_ps")

                def matmul(self, agg_ps, msgs, S, grp, sloc, par, start, stop):
                    nc.tensor.matmul(
                        agg_ps[:],
                        lhsT=S[:, sloc * P:(sloc + 1) * P],
                        rhs=msgs[:, grp * 2 * OUT + par * OUT:
                                 grp * 2 * OUT + (par + 1) * OUT],
                        start=start, stop=stop and not use_b2,
                    )

                def finish(self, b, agg_ps):
                    if agg_ps is None:
                        nc.vector.memset(rows_o[:, b * OUT:(b + 1) * OUT], 0.0)
                        return
                    if use_b2:
                        nc.tensor.matmul(
                            agg_ps[:],
                            lhsT=ivd_sb[:, b * P:(b + 1) * P],
                            rhs=b2_sb[:],
                            start=False, stop=True,
                        )
                    nc.vector.tensor_scalar(
                        rows_o[:, b * OUT:(b + 1) * OUT], agg_ps[:],
                        dv1_sb[:, b:b + 1], None, mul,
                    )

            run_layer(l2, tab2, GT // 2, idx2_sb, met2_sb, 2 * OUT,
                      Epi2Driver())

            nc.sync.dma_start(outt[:].rearrange("(b p) f -> p b f", p=P),
                              rows_o[:])

    nc.compile()
    return nc


# --------------------------------------------------------------------------
# optional NTFF tracing (dev only; registers the axon profile hook)
# --------------------------------------------------------------------------

def _install_trace_shim():
    try:
        if "antenv.axon_hooks" in sys.modules:
            return True
        import antenv

        mod = types.ModuleType("antenv.axon_hooks")
        mod._hook = None
        mod.set_axon_ntff_profile_hook = lambda h: setattr(mod, "_hook", h)
        mod.get_axon_ntff_profile_hook = lambda: mod._hook
        sys.modules["antenv.axon_hooks"] = mod
        antenv.axon_hooks = mod
        from trn_agent_boot.trn_boot import _ntff_profile_via_ctypes

        mod.set_axon_ntff_profile_hook(
            _ntff_profile_via_ctypes("/opt/axon/libaxon_pjrt.so")
        )
        import concourse.bass_utils as bu

        bu.upload_artifacts = lambda tmpdir: ""
        return True
    except Exception:
        return False


LAST_EXEC_NS = None
LAST_RESULTS = None


def kernel(x, edge_index, W1, b1, W2, b2):
    global LAST_EXEC_NS, LAST_RESULTS
    from concourse.bass_utils import run_bass_kernel_spmd

    x = np.asarray(x, dtype=np.float32)
    W1 = np.ascontiguousarray(np.asarray(W1, np.float32))
    b1 = np.asarray(b1, np.float32)
    W2 = np.ascontiguousarray(np.asarray(W2, np.float32))
    b2 = np.asarray(b2, np.float32)
    N, IN = x.shape
    HID = W1.shape[1]
    OUT = W2.shape[1]

    pp = _preprocess(x, edge_index)
    B, PADN, NPC = pp["B"], pp["PADN"], pp["NPC"]
    use_b1 = bool(np.any(b1))
    use_b2 = bool(np.any(b2))

    nc = _build(N, IN, HID, OUT, B, PADN, pp["l1"], pp["l2"],
                use_b1, use_b2)

    xs = (x * pp["dinv"][:, None].astype(np.float32)).astype(ml_dtypes.bfloat16)
    xs = np.ascontiguousarray(xs)
    newpos = pp["newpos"]
    w1b = W1.astype(ml_dtypes.bfloat16)
    w2b = W2.astype(ml_dtypes.bfloat16)
    b1b = b1.reshape(1, HID).astype(ml_dtypes.bfloat16)
    b2b = b2.reshape(1, OUT).astype(ml_dtypes.bfloat16)

    in_maps = []
    for c in range(NCORES):
        in_maps.append(
            {
                "xt": xs,
                "idx1": pp["l1"]["idx16"][c],
                "idx2": pp["l2"]["idx16"][c],
                "met1": pp["l1"]["met"][c],
                "met2": pp["l2"]["met"][c],
                "w1": w1b,
                "w2": w2b,
                "b1t": b1b,
                "b2t": b2b,
                "dv1": pp["dinv_blk"][c],
                "dv2": pp["dinv2_blk"][c],
                "ivd": pp["invd_row"][c],
            }
        )

    trace = bool(int(os.environ.get("GCN_TRACE", "0")))
    if trace:
        trace = _install_trace_shim()
    res = run_bass_kernel_spmd(
        nc, in_maps, core_ids=list(range(NCORES)), trace=trace
    )
    LAST_EXEC_NS = res.exec_time_ns
    LAST_RESULTS = res

    full = np.concatenate([res.results[c]["outt"] for c in range(NCORES)],
                          axis=0)
    return np.ascontiguousarray(full[newpos])


# revision 45
# speedup vs baseline: 1.5932x; 1.1155x over previous
"""2-layer GCN (PyG GCNConv semantics) on 8 Trainium2 NeuronCores.

Distribution: destination-node sharding (12500 nodes/core), edges
partitioned by dst; params replicated; layer-2 input exchanged via a
bf16 AllGather of per-core shards.

Key structure (per core, all matmuls bf16 into fp32 PSUM):
  - Symmetric normalization is factorized: the gather table holds
    dinv[src]-prescaled rows, scatter matrices S are pure 0/1 one-hots,
    and dinv[dst] factors are applied per dst-block in the epilogues
    (relu commutes with the positive diagonal scale; biases enter via
    rank-1 PE matmuls scaled by 1/dinv so results are exact).
  - Edges are grouped per (dst-block, 32768-row table chunk[, parity])
    and packed into 128-slot bins.  Per chunk, one idx stream covers
    all blocks, chopped into 1024-index dma_gather calls (bf16 rows,
    256B each); call tails use negative (skipped) indices.
  - S for the 8 bins of a call is built with ONE DVE tensor_tensor:
    iota pattern vs the per-slot dst_local column broadcast along free.
  - Layer 1: agg[in,dst] += msgsT @ S per bin; z[hid,dst] = W1^T agg
    (+ b1x(1/dinv) rank-1); h = relu (ScalarE, bf16); p[dst,out] =
    h^T W2; rows written x dinv^2 -> bf16 p_shard; AllGather.
  - Layer-2 table is the bf16 p matrix viewed as pair-packed rows
    [GT/2, 128] so gathers stay at the 256B descriptor minimum; bins
    are split by src parity and use the matching 64-wide half of msgs.
  - Layer 2: agg2[dst,out] += S^T @ msgs_half per bin (+ (1/dinv)xb2
    rank-1); written x dinv as fp32 output rows.  No PE transposes.
  - PSUM->SBUF moves ride the idle ScalarE; DVE only builds S and does
    the two per-block scaled writes.

kernel(**inputs) takes FULL inputs, returns the FULL [N, 64] f32 output.
Set GCN_TRACE=1 to capture an NTFF profile (exec time in LAST_EXEC_NS).
"""

import math
import os
import sys
import types

import numpy as np
import ml_dtypes

P = 128
NCORES = 8
CHUNK = 32768          # int16 index range limit per gather table chunk
CALL = 2048            # idxs per dma_gather call (16 bins)
BINS_PER_CALL = CALL // P


# --------------------------------------------------------------------------
# host-side preprocessing
# --------------------------------------------------------------------------

def _round128(v):
    return ((v + P - 1) // P) * P


def _plan_layer(trows, pars, dst_loc, blks, B, n_tab_rows, npar):
    """Build the uniform call/bin plan plus per-core idx/met arrays.

    trows[c]   : table row index per edge (int64)
    pars[c]    : parity (0..npar-1) per edge, selects the 64-wide half
    dst_loc[c] : dst % 128 per edge
    blks[c]    : dst block per edge
    Returns dict with idx16 [NCORES,128,icols], met [NCORES,128,nbins]
    (bf16), call list and per-block bin lists.
    """
    nchunks = (n_tab_rows + CHUNK - 1) // CHUNK
    cw = (n_tab_rows + nchunks - 1) // nchunks  # equal-width chunks
    G = nchunks * npar
    sizes = np.zeros((NCORES, B, G), np.int64)
    order_by_core = []
    bounds_by_core = []
    for c in range(NCORES):
        ch = trows[c] // cw
        key = blks[c] * G + ch * npar + pars[c]
        order = np.argsort(key, kind="stable")
        key_s = key[order]
        bounds = np.searchsorted(key_s, np.arange(B * G + 1))
        cnt = bounds[1:] - bounds[:-1]
        sizes[c] = cnt.reshape(B, G)
        order_by_core.append(order)
        bounds_by_core.append(bounds)

    # exact segments (no rounding): boundary bins span two segments and
    # get one scatter-matmul instance per segment touching them
    seg = sizes.max(axis=0)  # [B, G] caps

    S_k = [int(seg[:, k * npar:(k + 1) * npar].sum()) for k in range(nchunks)]
    ncalls_k = [(s + CALL - 1) // CALL for s in S_k]
    callbase = np.concatenate([[0], np.cumsum(ncalls_k)]).astype(np.int64)
    ncalls = int(callbase[-1])
    icols = ncalls * (CALL // 16)

    # stream offset of (b, g) within its chunk
    off = np.zeros((B, G), np.int64)
    run = [0] * nchunks
    for b in range(B):
        for k in range(nchunks):
            for p_ in range(npar):
                g = k * npar + p_
                off[b, g] = run[k]
                run[k] += int(seg[b, g])

    # instances: per (b, g) one per touched 128-slot group, in stream
    # order (== group order per chunk).  Assign met columns sequentially
    # per call.
    inst_of = {}              # (b, g) -> list of (J, grp_in_call, met_col)
    n_inst_call = [0] * ncalls
    inst0_call = [0] * ncalls
    per_chunk_insts = [[] for _ in range(nchunks)]
    for k in range(nchunks):
        items = []
        for b in range(B):
            for p_ in range(npar):
                g = k * npar + p_
                cap = int(seg[b, g])
                if cap == 0:
                    inst_of[(b, g)] = []
                    continue
                o = int(off[b, g])
                g0, g1 = o // P, (o + cap - 1) // P
                items.append((o, b, g, g0, g1))
        items.sort()
        for (o, b, g, g0, g1) in items:
            lst = []
            for grp in range(g0, g1 + 1):
                J = int(callbase[k]) + grp // BINS_PER_CALL
                lst.append([J, grp % BINS_PER_CALL, None])
                per_chunk_insts[k].append((grp, o, b, g, lst[-1]))
            inst_of[(b, g)] = lst
    # met columns: order instances by (call, then stream order)
    nmet = 0
    for k in range(nchunks):
        per_chunk_insts[k].sort(key=lambda t: (t[0], t[1]))
        for (grp, o, b, g, ref) in per_chunk_insts[k]:
            J = ref[0]
            n_inst_call[J] += 1
    for J in range(1, ncalls):
        inst0_call[J] = inst0_call[J - 1] + n_inst_call[J - 1]
    nmet = inst0_call[-1] + n_inst_call[-1] if ncalls else 0
    fill = [0] * ncalls
    for k in range(nchunks):
        for (grp, o, b, g, ref) in per_chunk_insts[k]:
            J = ref[0]
            ref[2] = inst0_call[J] + fill[J]
            fill[J] += 1

    calls = []
    for k in range(nchunks):
        for j in range(ncalls_k[k]):
            J = int(callbase[k]) + j
            # every slot is gathered (tail pads hit row 0) so no SBUF
            # garbage ever reaches the PE (0 x NaN would poison PSUM)
            calls.append(dict(k=k, J=J, col0=J * (CALL // 16), v=CALL,
                              inst0=inst0_call[J], n_inst=n_inst_call[J]))

    # per-block instance list in stream order
    blocks = []
    for b in range(B):
        bl = []
        for k in range(nchunks):
            for p_ in range(npar):
                g = k * npar + p_
                for (J, grp, col) in inst_of[(b, g)]:
                    bl.append((J, grp, col, p_))
        blocks.append(bl)

    # fill idx16 / met
    idx16 = np.zeros((NCORES, 16, max(icols, 1)), np.int16)
    met = np.full((NCORES, P, max(nmet, 1)), -1.0, ml_dtypes.bfloat16)
    for c in range(NCORES):
        order = order_by_core[c]
        bounds = bounds_by_core[c]
        tr = trows[c]
        dl = dst_loc[c]
        for b in range(B):
            for k in range(nchunks):
                for p_ in range(npar):
                    g = k * npar + p_
                    s0, s1 = bounds[b * G + g], bounds[b * G + g + 1]
                    n = s1 - s0
                    cap = int(seg[b, g])
                    if cap == 0:
                        continue
                    e = order[s0:s1]
                    if n > 1:
                        e = e[np.argsort(tr[e], kind="stable")]
                    o = int(off[b, g])
                    iv = np.zeros(cap, np.int64)  # idx values (pad -> 0)
                    if n:
                        iv[:n] = tr[e] - k * cw
                    pos = o + np.arange(cap)
                    idx16[c, pos % 16, int(callbase[k]) * (CALL // 16)
                          + pos // 16] = iv.astype(np.int16)
                    mv = np.full(cap, -1.0, np.float32)
                    if n:
                        mv[:n] = dl[e]
                    # met column of slot = instance of its group
                    g0 = o // P
                    cols = np.array([ic[2] for ic in inst_of[(b, g)]],
                                    np.int64)
                    met[c, pos % P, cols[pos // P - g0]] = \
                        mv.astype(ml_dtypes.bfloat16)
    # call tails beyond valid stay -1 (skipped by HW); mid-stream pads are 0
    idx_full = np.empty((NCORES, P, max(icols, 1)), np.int16)
    for gsh in range(8):
        idx_full[:, gsh * 16:(gsh + 1) * 16, :] = idx16
    return dict(calls=calls, blocks=blocks, icols=max(icols, 1),
                nbins=max(nmet, 1), idx16=idx_full, met=met,
                nchunks=nchunks, npar=npar, cw=cw)


def _preprocess(x, edge_index):
    N = x.shape[0]
    # The implicit self-loops (reference appends arange(N) to the edge
    # list) are handled per dst block with one identity matmul over
    # locally-available rows; only the explicit edges are gathered.
    src = np.asarray(edge_index[0]).astype(np.int64)
    dst = np.asarray(edge_index[1]).astype(np.int64)
    deg = (np.bincount(dst, minlength=N) + 1).astype(np.float64)
    dinv = 1.0 / np.sqrt(deg)

    assert N % NCORES == 0
    NPC = N // NCORES
    B = (NPC + P - 1) // P
    PADN = B * P
    GT = NCORES * PADN
    NB = NCORES * B

    # Degree-balanced dst assignment: deal nodes round-robin (by degree
    # rank) across the NCORES*B (core, block) bins so per-bin edge counts
    # are nearly equal across cores -> minimal cap-max padding.
    order = np.argsort(-deg, kind="stable")
    newpos = np.empty(N, np.int64)
    ranks = np.arange(N)
    newpos[order] = (ranks % NB) * P + ranks // NB
    assert newpos.max() < GT

    dnew = newpos[dst]
    core_of = dnew // PADN
    trows1, trows2, pars2, dstls, blks = [], [], [], [], []
    for c in range(NCORES):
        m = core_of == c
        se = src[m]
        dl = dnew[m] - c * PADN
        blks.append(dl // P)
        dstls.append((dl % P).astype(np.float32))
        spad = newpos[se]
        trows1.append(spad)  # x table stored in permuted order
        trows2.append(spad >> 1)
        pars2.append((spad & 1).astype(np.int64))

    zeros = [np.zeros_like(t) for t in trows1]
    l1 = _plan_layer(trows1, zeros, dstls, blks, B, GT, 1)
    l2 = _plan_layer(trows2, pars2, dstls, blks, B, GT // 2, 2)

    # per-core dinv vectors in the permuted padded layout [128, B]
    dinv_pad = np.zeros(GT)
    dinv_pad[newpos] = dinv
    dinv_blk = np.zeros((NCORES, P, B), np.float32)
    dinv2_blk = np.zeros((NCORES, P, B), np.float32)
    invd_row = np.zeros((NCORES, 1, PADN), ml_dtypes.bfloat16)
    for c in range(NCORES):
        pad = dinv_pad[c * PADN:(c + 1) * PADN]
        dinv_blk[c] = pad.reshape(B, P).T.astype(np.float32)
        dinv2_blk[c] = (pad ** 2).reshape(B, P).T.astype(np.float32)
        iv = np.where(pad > 0, 1.0 / np.maximum(pad, 1e-30), 0.0)
        invd_row[c, 0] = iv.astype(ml_dtypes.bfloat16)

    return dict(NPC=NPC, B=B, PADN=PADN, l1=l1, l2=l2, dinv=dinv,
                newpos=newpos, dinv_blk=dinv_blk, dinv2_blk=dinv2_blk,
                invd_row=invd_row)


# --------------------------------------------------------------------------
# bass program
# --------------------------------------------------------------------------

def _build(N, IN, HID, OUT, B, PADN, l1, l2, use_b1, use_b2,
           use_collective=True):
    import concourse.bass as bass
    import concourse.bacc as bacc
    import concourse.mybir as mybir
    import concourse.tile as tile

    f32 = mybir.dt.float32
    bf16 = mybir.dt.bfloat16
    i16 = mybir.dt.int16
    i32 = mybir.dt.int32
    eq = mybir.AluOpType.is_equal
    mul = mybir.AluOpType.mult
    Copy = mybir.ActivationFunctionType.Copy
    Relu = mybir.ActivationFunctionType.Relu
    GT = NCORES * PADN

    nc = bacc.Bacc("TRN2", num_devices=NCORES)
    xt = nc.dram_tensor("xt", [NCORES * PADN, IN], bf16, kind="ExternalInput")
    xso = nc.dram_tensor("xso", [PADN, IN], bf16, kind="ExternalInput")
    idx1 = nc.dram_tensor("idx1", [P, l1["icols"]], i16, kind="ExternalInput")
    idx2 = nc.dram_tensor("idx2", [P, l2["icols"]], i16, kind="ExternalInput")
    met1 = nc.dram_tensor("met1", [P, l1["nbins"]], bf16, kind="ExternalInput")
    met2 = nc.dram_tensor("met2", [P, l2["nbins"]], bf16, kind="ExternalInput")
    w1 = nc.dram_tensor("w1", [IN, HID], bf16, kind="ExternalInput")
    w2 = nc.dram_tensor("w2", [HID, OUT], bf16, kind="ExternalInput")
    b1t = nc.dram_tensor("b1t", [1, HID], bf16, kind="ExternalInput")
    b2t = nc.dram_tensor("b2t", [1, OUT], bf16, kind="ExternalInput")
    dv1 = nc.dram_tensor("dv1", [P, B], f32, kind="ExternalInput")
    dv2 = nc.dram_tensor("dv2", [P, B], f32, kind="ExternalInput")
    ivd = nc.dram_tensor("ivd", [1, PADN], bf16, kind="ExternalInput")
    p_shard = nc.dram_tensor("p_shard", [PADN, OUT], bf16, kind="Internal")
    if use_collective:
        p_full = nc.dram_tensor("p_full", [GT, OUT], bf16, kind="Internal",
                                addr_space="Shared")
    else:
        p_full = nc.dram_tensor("p_full", [GT, OUT], bf16, kind="Internal")
    outt = nc.dram_tensor("outt", [PADN, OUT], f32, kind="ExternalOutput")

    with tile.TileContext(nc) as tc:
        with (
            tc.tile_pool(name="const", bufs=1) as cpool,
            tc.tile_pool(name="meta", bufs=1) as mpool,
            tc.tile_pool(name="gath", bufs=6) as gpool,
            tc.tile_pool(name="smat", bufs=6) as spool,
            tc.tile_pool(name="work", bufs=4) as wpool,
            tc.tile_pool(name="psA", bufs=2, space="PSUM") as psA,
            tc.tile_pool(name="psB", bufs=2, space="PSUM") as psB,
            tc.tile_pool(name="psC", bufs=2, space="PSUM") as psC,
        ):
            w1_sb = cpool.tile([IN, HID], bf16)
            nc.sync.dma_start(w1_sb[:], w1[:])
            w2_sb = cpool.tile([HID, OUT], bf16)
            nc.sync.dma_start(w2_sb[:], w2[:])
            b1_sb = cpool.tile([1, HID], bf16)
            nc.sync.dma_start(b1_sb[:], b1t[:])
            b2_sb = cpool.tile([1, OUT], bf16)
            nc.sync.dma_start(b2_sb[:], b2t[:])
            dv1_sb = cpool.tile([P, B], f32)
            nc.sync.dma_start(dv1_sb[:], dv1[:])
            dv2_sb = cpool.tile([P, B], f32)
            nc.sync.dma_start(dv2_sb[:], dv2[:])
            ivd_sb = cpool.tile([1, PADN], bf16)
            nc.sync.dma_start(ivd_sb[:], ivd[:])

            maxi = max(max(c["n_inst"] for c in l1["calls"]),
                       max(c["n_inst"] for c in l2["calls"]))
            iota_i = cpool.tile([P, maxi * P], i32)
            nc.gpsimd.iota(iota_i[:], pattern=[[0, maxi], [1, P]],
                           base=0, channel_multiplier=0)
            iota8 = cpool.tile([P, maxi * P], bf16)
            nc.vector.tensor_copy(iota8[:], iota_i[:])
            # identity (bf16) for the per-block self-loop contribution
            iotap_i = cpool.tile([P, P], i32)
            nc.gpsimd.iota(iotap_i[:], pattern=[[0, P]], base=0,
                           channel_multiplier=1)
            iotap = cpool.tile([P, P], bf16)
            nc.vector.tensor_copy(iotap[:], iotap_i[:])
            ident = cpool.tile([P, P], bf16)
            nc.vector.tensor_tensor(ident[:], iota8[:, :P], iotap[:], eq)

            idx1_sb = mpool.tile([P, l1["icols"]], i16)
            nc.sync.dma_start(idx1_sb[:], idx1[:])
            met1_sb = mpool.tile([P, l1["nbins"]], bf16)
            nc.sync.dma_start(met1_sb[:], met1[:])
            idx2_sb = mpool.tile([P, l2["icols"]], i16)
            nc.sync.dma_start(idx2_sb[:], idx2[:])
            met2_sb = mpool.tile([P, l2["nbins"]], bf16)
            nc.sync.dma_start(met2_sb[:], met2[:])

            rows_p = mpool.tile([P, B * OUT], bf16)
            rows_o = mpool.tile([P, B * OUT], f32)

            def run_layer(lp, table_ap, tab_rows, idx_sb, met_sb, elem,
                          epilogue):
                """Emit gathers/S-builds on demand and per-block matmul
                accumulation; epilogue(b, agg_ps) per block."""
                tiles = {}     # J -> (msgs_tile, S_tile)
                emitted = [0] * lp["nchunks"]
                callbase = {}
                by_J = {}
                for cinfo in lp["calls"]:
                    callbase.setdefault(cinfo["k"], []).append(cinfo)
                    by_J[cinfo["J"]] = cinfo

                def emit_call(k, jloc):
                    cinfo = callbase[k][jloc]
                    J = cinfo["J"]
                    lo = k * lp["cw"]
                    hi = min(lo + lp["cw"], tab_rows)
                    ni = cinfo["n_inst"]
                    msgs = gpool.tile([P, BINS_PER_CALL * elem], bf16,
                                      tag="msgs")
                    nc.gpsimd.dma_gather(
                        out_ap=msgs[:].rearrange("p (s e) -> p s e", e=elem),
                        in_ap=table_ap[lo:hi],
                        idxs_ap=idx_sb[:, cinfo["col0"]:cinfo["col0"] + CALL // 16],
                        num_idxs=CALL,
                        num_idxs_reg=cinfo["v"],
                        elem_size=elem,
                        single_packet=False,
                    )
                    S = spool.tile([P, ni * P], bf16, tag="S")
                    met_b = met_sb[:, cinfo["inst0"]:cinfo["inst0"] + ni]
                    nc.vector.tensor_tensor(
                        S[:].rearrange("p (k f) -> p k f", f=P),
                        iota8[:, :ni * P].rearrange("p (k f) -> p k f", f=P),
                        met_b.unsqueeze(2).broadcast_to([P, ni, P]),
                        eq,
                    )
                    tiles[J] = (msgs, S)

                for b in range(B):
                    bl = lp["blocks"][b]
                    nb = len(bl)
                    agg_ps = epilogue.begin(b, has_more=nb > 0)
                    for i, (J, grp, col, par) in enumerate(bl):
                        cinfo = by_J[J]
                        k = cinfo["k"]
                        jloc = J - callbase[k][0]["J"]
                        while emitted[k] <= jloc:
                            emit_call(k, emitted[k])
                            emitted[k] += 1
                        msgs, S = tiles[J]
                        epilogue.matmul(agg_ps, msgs, S, grp,
                                        col - cinfo["inst0"], par,
                                        start=False, stop=(i == nb - 1))
                    epilogue.finish(b, agg_ps)

            # ---------------- layer 1 ----------------
            class Epi1:
                def begin(self, b, has_more):
                    agg_ps = psA.tile([IN, P], f32, tag="agg", name="agg_ps")
                    xblk = wpool.tile([P, IN], bf16, tag="xblk")
                    nc.sync.dma_start(xblk[:], xso[b * P:(b + 1) * P, :])
                    nc.tensor.matmul(agg_ps[:], lhsT=xblk[:], rhs=ident[:],
                                     start=True, stop=not has_more)
                    return agg_ps

                def matmul(self, agg_ps, msgs, S, grp, sloc, par, start, stop):
                    nc.tensor.matmul(
                        agg_ps[:],
                        lhsT=msgs[:, grp * IN:(grp + 1) * IN],
                        rhs=S[:, sloc * P:(sloc + 1) * P],
                        start=start, stop=stop,
                    )

                def finish(self, b, agg_ps):
                    agg_sb = wpool.tile([IN, P], bf16, tag="aggsb")
                    nc.scalar.activation(agg_sb[:], agg_ps[:], Copy)
                    z_ps = psB.tile([HID, P], f32, tag="z")
                    nc.tensor.matmul(z_ps[:], lhsT=w1_sb[:], rhs=agg_sb[:],
                                     start=True, stop=not use_b1)
                    if use_b1:
                        nc.tensor.matmul(
                            z_ps[:], lhsT=b1_sb[:],
                            rhs=ivd_sb[:, b * P:(b + 1) * P],
                            start=False, stop=True,
                        )
                    h_sb = wpool.tile([HID, P], bf16, tag="h")
                    nc.scalar.activation(h_sb[:], z_ps[:], Relu)
                    p_ps = psC.tile([P, OUT], f32, tag="p")
                    nc.tensor.matmul(p_ps[:], lhsT=h_sb[:], rhs=w2_sb[:],
                                     start=True, stop=True)
                    nc.vector.tensor_scalar(
                        rows_p[:, b * OUT:(b + 1) * OUT], p_ps[:],
                        dv2_sb[:, b:b + 1], None, mul,
                    )

            run_layer(l1, xt[:], GT, idx1_sb, met1_sb, IN, Epi1())

            nc.sync.dma_start(p_shard[:].rearrange("(b p) f -> p b f", p=P),
                              rows_p[:])
            if use_collective:
                nc.gpsimd.collective_compute(
                    "AllGather",
                    mybir.AluOpType.bypass,
                    replica_groups=[list(range(NCORES))],
                    ins=[p_shard[:]],
                    outs=[p_full[:]],
                )
            else:
                nc.sync.dma_start(p_full[0:PADN, :], p_shard[:])

            # ---------------- layer 2 ----------------
            tab2 = p_full[:].rearrange("(r two) f -> r (two f)", two=2)

            class Epi2Driver:
                def begin(self, b, has_more):
                    agg_ps = psC.tile([P, OUT], f32, tag="agg2",
                                      name="agg2_ps")
                    nc.tensor.matmul(
                        agg_ps[:], lhsT=ident[:],
                        rhs=rows_p[:, b * OUT:(b + 1) * OUT],
                        start=True, stop=not has_more and not use_b2,
                    )
                    return agg_ps

                def matmul(self, agg_ps, msgs, S, grp, sloc, par, start, stop):
                    nc.tensor.matmul(
                        agg_ps[:],
                        lhsT=S[:, sloc * P:(sloc + 1) * P],
                        rhs=msgs[:, grp * 2 * OUT + par * OUT:
                                 grp * 2 * OUT + (par + 1) * OUT],
                        start=start, stop=stop and not use_b2,
                    )

                def finish(self, b, agg_ps):
                    if use_b2:
                        nc.tensor.matmul(
                            agg_ps[:],
                            lhsT=ivd_sb[:, b * P:(b + 1) * P],
                            rhs=b2_sb[:],
                            start=False, stop=True,
                        )
                    nc.vector.tensor_scalar(
                        rows_o[:, b * OUT:(b + 1) * OUT], agg_ps[:],
                        dv1_sb[:, b:b + 1], None, mul,
                    )

            run_layer(l2, tab2, GT // 2, idx2_sb, met2_sb, 2 * OUT,
                      Epi2Driver())

            nc.sync.dma_start(outt[:].rearrange("(b p) f -> p b f", p=P),
                              rows_o[:])

    nc.compile()
    return nc


# --------------------------------------------------------------------------
# optional NTFF tracing (dev only; registers the axon profile hook)
# --------------------------------------------------------------------------

def _install_trace_shim():
    try:
        if "antenv.axon_hooks" in sys.modules:
            return True
        import antenv

        mod = types.ModuleType("antenv.axon_hooks")
        mod._hook = None
        mod.set_axon_ntff_profile_hook = lambda h: setattr(mod, "_hook", h)
        mod.get_axon_ntff_profile_hook = lambda: mod._hook
        sys.modules["antenv.axon_hooks"] = mod
        antenv.axon_hooks = mod
        from trn_agent_boot.trn_boot import _ntff_profile_via_ctypes

        mod.set_axon_ntff_profile_hook(
            _ntff_profile_via_ctypes("/opt/axon/libaxon_pjrt.so")
        )
        import concourse.bass_utils as bu

        bu.upload_artifacts = lambda tmpdir: ""
        return True
    except Exception:
        return False


LAST_EXEC_NS = None
LAST_RESULTS = None


def kernel(x, edge_index, W1, b1, W2, b2):
    global LAST_EXEC_NS, LAST_RESULTS
    from concourse.bass_utils import run_bass_kernel_spmd

    x = np.asarray(x, dtype=np.float32)
    W1 = np.ascontiguousarray(np.asarray(W1, np.float32))
    b1 = np.asarray(b1, np.float32)
    W2 = np.ascontiguousarray(np.asarray(W2, np.float32))
    b2 = np.asarray(b2, np.float32)
    N, IN = x.shape
    HID = W1.shape[1]
    OUT = W2.shape[1]

    pp = _preprocess(x, edge_index)
    B, PADN, NPC = pp["B"], pp["PADN"], pp["NPC"]
    use_b1 = bool(np.any(b1))
    use_b2 = bool(np.any(b2))

    nc = _build(N, IN, HID, OUT, B, PADN, pp["l1"], pp["l2"],
                use_b1, use_b2)

    newpos = pp["newpos"]
    xs = (x * pp["dinv"][:, None].astype(np.float32)).astype(ml_dtypes.bfloat16)
    xs_perm = np.zeros((NCORES * PADN, IN), ml_dtypes.bfloat16)
    xs_perm[newpos] = xs
    w1b = W1.astype(ml_dtypes.bfloat16)
    w2b = W2.astype(ml_dtypes.bfloat16)
    b1b = b1.reshape(1, HID).astype(ml_dtypes.bfloat16)
    b2b = b2.reshape(1, OUT).astype(ml_dtypes.bfloat16)

    in_maps = []
    for c in range(NCORES):
        in_maps.append(
            {
                "xt": xs_perm,
                "xso": np.ascontiguousarray(xs_perm[c * PADN:(c + 1) * PADN]),
                "idx1": pp["l1"]["idx16"][c],
                "idx2": pp["l2"]["idx16"][c],
                "met1": pp["l1"]["met"][c],
                "met2": pp["l2"]["met"][c],
                "w1": w1b,
                "w2": w2b,
                "b1t": b1b,
                "b2t": b2b,
                "dv1": pp["dinv_blk"][c],
                "dv2": pp["dinv2_blk"][c],
                "ivd": pp["invd_row"][c],
            }
        )

    trace = bool(int(os.environ.get("GCN_TRACE", "0")))
    if trace:
        trace = _install_trace_shim()
    res = run_bass_kernel_spmd(
        nc, in_maps, core_ids=list(range(NCORES)), trace=trace
    )
    LAST_EXEC_NS = res.exec_time_ns
    LAST_RESULTS = res

    full = np.concatenate([res.results[c]["outt"] for c in range(NCORES)],
                          axis=0)
    return np.ascontiguousarray(full[newpos])
